# revision 23
# baseline (speedup 1.0000x reference)
"""NodeVarGraphConvolutionLayer on 8 TRN2 NeuronCores.

Math (see reference):
  Xs = X.sum(-1)                        [B, N]
  P0 = Xs;  P_i = A @ P_{i-1}           (3 batched matvecs, N=1024)
  Y[b,n,c] = sum_i h[i,c,n] * P_i[b,n]  [B, N, 64]
  out = tanh(LayerNorm_c(Y) * gamma + beta)

Sharding: data-parallel over batch. B=16 -> 2 batches per core.

Design (stationary-A, fp16):
  Precision: bf16 fails the 2e-2 gate (LayerNorm rescales nodes whose
  matvec cancels to O(1), so dot noise flips signs -> rel 5.5e-2); fp16
  with fp32 PSUM accumulation lands at rel ~9e-3. P_i grows ~30x per
  term, so pass i stores P_i * 16^-i (ACT copy, scale=1/16) and the
  matching 16^i is folded into h on the host.
  Matvec: A is the PE *stationary* operand. For chunk j, block u:
      stat[p, s] = AT[128j+p, 128u+s],  mov[p] = P_{i-1}[128j+p]
      => out[s] = P_i[128u+s] accumulated over j into PSUM bank u
  so results land directly in column layout (n = 128t + p) - no
  transpose pipeline. A matmul start zeroes a whole 2KB PSUM bank, so
  the 8 interleaved groups own one bank each (pt_all spans all 8).
  PE cost ~ the A-stream floor: 64 self-loading (128-row ldweights + 1
  moving row) tiles per pass.
  Xs is also computed on the PE (X^T tiles x ones), directly in column
  layout.
  Schedule: batch-sequential passes (b0: P1..P3 then b1) so only b0's
  pass 1 is DMA-gated and all of b0's epilogue hides under b1's
  matmuls. Queues: A chunks + OUT stores on SP hwdge, X^T on ACT hwdge,
  HS|G + h on the Pool SWDGE queue (the ACT SEQ's 1.2us-per-DMA issue
  cost would otherwise sit in front of the first col copy). The
  pass-gating colb copies run on DVE (lower PSUM latency, and off the
  tanh-heavy ACT queue); colf/tanh on ACT. Emission order is arranged
  so no in-order queue blocks a later-needed op behind a long wait.
  Epilogue: Yacc (+)= h_i*P_i per pass in the next pass's shadow (b0 and
  b1's last term on Pool, b1's others on DVE). LayerNorm stats come from
  host-precomputed Gram vectors of the quantized h
  (S1 = sum_i hs_i P_i, S2 = sum_ij G_ij P_i P_j, both tiny DVE ops off
  the stored P vectors) - they never wait on Yacc assembly. rstd =
  Quake seed + 1 Newton step; tanh is fused per-t on ACT via
  scale/bias (tanh(Y*rstd - mean*rstd)); fp16 output (host upcasts).
"""

import numpy as np

B, N, C, K1 = 16, 1024, 64, 4
NCORES = 8
BPC = B // NCORES  # batches per core
LN_EPS = 1e-5
SCALE = 16.0

_NC = None


def _build_module():
    from concourse import bacc, bass, tile, mybir

    f32 = mybir.dt.float32
    f16 = mybir.dt.float16
    i32 = mybir.dt.int32
    AX = mybir.AxisListType
    OP = mybir.AluOpType
    AF = mybir.ActivationFunctionType

    nc = bacc.Bacc(
        "TRN2",
        target_bir_lowering=False,
        debug=False,
        enable_asserts=False,
    )

    AT_d = nc.dram_tensor("AT", [BPC, N, N], f16, kind="ExternalInput").ap()
    XT_d = nc.dram_tensor("XT", [BPC, C, N], f16, kind="ExternalInput").ap()
    HT_d = nc.dram_tensor("HT", [N, C * K1], f16, kind="ExternalInput").ap()
    # HS|G packed, already in column layout [p, t, 4+16]
    HSG_d = nc.dram_tensor("HSG", [128, 8, K1 + K1 * K1], f32, kind="ExternalInput").ap()
    OUT_d = nc.dram_tensor("OUT", [BPC, N, C], f16, kind="ExternalOutput").ap()

    with tile.TileContext(nc) as tc:
        with (
            tc.tile_pool(name="big", bufs=2) as big,
            tc.tile_pool(name="aux", bufs=1) as aux,
            tc.tile_pool(name="psum", bufs=1, space="PSUM") as psum,
        ):
            # --- small loads on the ACT hwdge queue ----------------------
            XT_sbs = []
            for b in range(BPC):
                XT_sb = aux.tile([C, N], f16, tag=f"XT{b}", name=f"XT_sb{b}")
                nc.scalar.dma_start(XT_sb, XT_d[b])
                XT_sbs.append(XT_sb)
            # LN-stat vectors + h on the Pool SWDGE queue so the ACT SEQ
            # only issues the two X^T loads before the first col copies.
            HSG_sb = aux.tile([128, 8, K1 + K1 * K1], f32, tag="HSG")
            nc.gpsimd.dma_start(HSG_sb, HSG_d)
            # view [p, t, 5, 4]: row 0 = hs, rows 1..4 = G
            HSGv = HSG_sb.rearrange("p t (i j) -> p t i j", i=K1 + 1)
            H_sb = aux.tile([128, 8, C * K1], f16, tag="H")
            nc.gpsimd.dma_start(H_sb, HT_d.rearrange("(t p) x -> p t x", t=8, p=128))
            Hv = H_sb.rearrange("p t (c i) -> p t c i", c=C, i=K1)

            zero_sb = aux.tile([128, 1], f32, tag="zero")
            nc.vector.memset(zero_sb, 0.0)
            magic = aux.tile([128, 1], i32, tag="magic")
            nc.vector.memset(magic, 0x5F3759DF)
            magicb = magic.broadcast_to([128, 8])
            ones_sb = aux.tile([C, 1], f16, tag="ones")
            nc.vector.memset(ones_sb, 1.0)
            wz = aux.tile([128, 128], f16, tag="wz")
            nc.vector.memset(wz, 0.0)
            # Preload the Tanh ACT table while DMAs run.
            warm = aux.tile([128, 1], f32, tag="warm")
            nc.scalar.activation(warm, zero_sb, AF.Tanh, bias=zero_sb)

            # pt_all spans all 8 PSUM banks: a matmul start zeroes a whole
            # 2KB bank region, so each interleaved group owns a bank.
            pt_all = psum.tile([128, 8, 512], f32, tag="pt", name="pt")
            # PE p-state warmup under the initial DMA window.
            for _ in range(16):
                nc.tensor.matmul(
                    pt_all[:, 0, 0:1], wz, wz[:, 0:1], start=True, stop=True
                )

            # --- A: 16 chunk DMAs on the SP hwdge queue ------------------
            # A_sb[b][j][p, m] = AT[b, 128j+p, m]
            # b0: 8 chunk DMAs (pass 1 streams them as they land); b1: one
            # big DMA (pass 1 for b1 starts late enough anyway) to cut
            # SEQ/HWDGE issue overhead.
            A0_sbs = [
                aux.tile([128, N], f16, tag=f"A0j{j}", name=f"A0j{j}")
                for j in range(8)
            ]
            for j in range(8):
                nc.sync.dma_start(A0_sbs[j], AT_d[0, 128 * j : 128 * (j + 1), :])
            A1_sb = aux.tile([128, 8, N], f16, tag="A1", name="A1_sb")
            for h in range(2):
                nc.sync.dma_start(
                    A1_sb[:, 4 * h : 4 * h + 4, :],
                    AT_d[1, 512 * h : 512 * h + 512, :].rearrange(
                        "(j p) m -> p j m", j=4, p=128
                    ),
                )
            A_sbs = [A0_sbs, [A1_sb[:, j, :] for j in range(8)]]

            # colb[p, t, i] = P_i[128t+p] * 16^-i (fp16; PE rhs + h-product
            # input); colf = f32 upcast of colb for the LN-stat ops.
            colbs, colfs, Yaccs = [], [], []
            for b in range(BPC):
                colbs.append(
                    big.tile([128, 8, K1], f16, tag=f"colb{b}", name=f"colb{b}")
                )
                colfs.append(
                    big.tile([128, 8, K1], f32, tag=f"colf{b}", name=f"colf{b}")
                )
                Yaccs.append(
                    big.tile([128, 8, C], f32, tag=f"Yacc{b}", name=f"Yacc{b}")
                )

            def xsum(b):
                # P_0 = X.sum(-1) on the PE: out[s] = sum_c XT[c, 128u+s],
                # straight into column layout.
                for u in range(8):
                    nc.tensor.matmul(
                        pt_all[:, u, 0:1],
                        XT_sbs[b][:, 128 * u : 128 * (u + 1)],
                        ones_sb,
                        start=True,
                        stop=True,
                    )
                nc.vector.tensor_scalar_mul(colbs[b][:, :, 0], pt_all[:, :, 0], 1.0)
                nc.scalar.copy(colfs[b][:, :, 0], colbs[b][:, :, 0])

            def matvec(b, i):
                colb = colbs[b]
                for j in range(8):
                    A_j = A_sbs[b][j]
                    mov = colb[:, j, i - 1 : i]
                    for u in range(8):
                        nc.tensor.matmul(
                            pt_all[:, u, 0:1],
                            A_j[:, 128 * u : 128 * (u + 1)],
                            mov,
                            start=(j == 0),
                            stop=(j == 7),
                        )
                if i == K1 - 1:
                    # last pass: nothing gates the PE afterwards - put colf
                    # (which feeds the LN-stat chain, the tail's pole)
                    # straight from PSUM on DVE and colb on ACT in parallel.
                    nc.vector.tensor_scalar_mul(
                        colfs[b][:, :, i], pt_all[:, :, 0], 1.0 / SCALE
                    )
                    nc.scalar.mul(colb[:, :, i], pt_all[:, :, 0], 1.0 / SCALE)
                else:
                    # colb gates the next pass (PSUM WAR): do it on DVE, off
                    # the tanh-heavy ACT queue and with DVE's lower PSUM
                    # latency; colf reads colb (SBUF) on ACT, trailing freely.
                    nc.vector.tensor_scalar_mul(
                        colb[:, :, i], pt_all[:, :, 0], 1.0 / SCALE
                    )
                    nc.scalar.copy(colfs[b][:, :, i], colb[:, :, i])

            def accum(b, i, engs):
                # Yacc[:, t, c] (+)= h_i[c, n] * P_i[n]; one engine per
                # t-half (pass the same engine twice to run full-width).
                colb = colbs[b]
                Yacc = Yaccs[b]
                for half, eng in enumerate(engs):
                    sl = slice(4 * half, 4 * half + 4)
                    pb = colb[:, sl, i].unsqueeze(2).broadcast_to([128, 4, C])
                    if i == 0:
                        eng.tensor_tensor(Yacc[:, sl], Hv[:, sl, :, 0], pb, OP.mult)
                    else:
                        tmpE = big.tile(
                            [128, 8, C],
                            f32,
                            tag=f"tmpE{b}i{i}",
                            name=f"tmpE{b}i{i}h{half}",
                        )
                        eng.tensor_tensor(tmpE[:, sl], Hv[:, sl, :, i], pb, OP.mult)
                        eng.tensor_tensor(Yacc[:, sl], Yacc[:, sl], tmpE[:, sl], OP.add)

            def sops(b):
                # LN stats straight from the stored P vectors, via one
                # augmented-Gram product GPa[r, j] = HSGv[r, j] * P_j:
                #   S1 = sum_j GPa[0, j]           (hs row)
                #   S2 = sum_ij GPa[1+i, j] * P_i  (Gram rows)
                colf = colfs[b]
                GPa = big.tile(
                    [128, 8, K1 + 1, K1], f32, tag=f"GPa{b}", name=f"GPa{b}"
                )
                nc.vector.tensor_tensor(
                    GPa,
                    HSGv,
                    colf.unsqueeze(2).broadcast_to([128, 8, K1 + 1, K1]),
                    OP.mult,
                )
                S1 = big.tile([128, 8], f32, tag=f"S1{b}", name=f"S1{b}")
                nc.vector.tensor_reduce(S1, GPa[:, :, 0, :], AX.X, OP.add)
                PP = big.tile([128, 8, K1, K1], f32, tag=f"PP{b}", name=f"PP{b}")
                nc.vector.tensor_tensor(
                    PP,
                    GPa[:, :, 1:, :],
                    colf.unsqueeze(3).broadcast_to([128, 8, K1, K1]),
                    OP.mult,
                )
                S2 = big.tile([128, 8], f32, tag=f"S2{b}", name=f"S2{b}")
                nc.vector.tensor_reduce(S2, PP, AX.XY, OP.add)
                return S1, S2

            def chain_tanh_store(b, S1, S2):
                # mean/var -> rstd (Quake + 1 Newton) -> fused tanh -> store
                # veps = S2/64 + eps - (S1/64)^2, with the mean folded into
                # the nmr op so it is off the rstd critical path
                mse = big.tile([128, 8], f32, tag=f"mse{b}", name=f"mse{b}")
                nc.vector.tensor_scalar(mse, S2, 1.0 / C, LN_EPS, OP.mult, OP.add)
                m2 = big.tile([128, 8], f32, tag=f"m2{b}", name=f"m2{b}")
                nc.vector.tensor_tensor(m2, S1, S1, OP.mult)
                veps = big.tile([128, 8], f32, tag=f"veps{b}", name=f"veps{b}")
                nc.vector.scalar_tensor_tensor(
                    veps, m2, -1.0 / (C * C), mse, OP.mult, OP.add
                )

                rstd = big.tile([128, 8], f32, tag=f"rstd{b}", name=f"rstd{b}")
                nc.vector.tensor_scalar(
                    rstd.bitcast(i32),
                    veps.bitcast(i32),
                    1,
                    None,
                    OP.logical_shift_right,
                )
                nc.vector.tensor_tensor(
                    rstd.bitcast(i32), magicb, rstd.bitcast(i32), OP.subtract
                )
                tq = big.tile([128, 8], f32, tag=f"tq{b}", name=f"tq{b}")
                nc.vector.tensor_tensor(tq, rstd, rstd, OP.mult)
                nc.vector.scalar_tensor_tensor(
                    tq, tq, -0.5, veps, OP.mult, OP.mult
                )
                nc.vector.scalar_tensor_tensor(
                    rstd, tq, 1.5, rstd, OP.add, OP.mult
                )
                nmr = big.tile([128, 8], f32, tag=f"nmr{b}", name=f"nmr{b}")
                nc.vector.scalar_tensor_tensor(
                    nmr, S1, -1.0 / C, rstd, OP.mult, OP.mult
                )

                # tanh(Yacc*rstd + nmr) per t on ACT; store halves on SP
                Yacc = Yaccs[b]
                OUT_sb = big.tile([128, 8, C], f16, tag=f"OUTS{b}", name=f"OUTS{b}")
                outv = OUT_d[b].rearrange("(t p) c -> p t c", t=8, p=128)
                for half in range(2):
                    for t in range(4 * half, 4 * half + 4):
                        nc.scalar.activation(
                            OUT_sb[:, t],
                            Yacc[:, t],
                            AF.Tanh,
                            bias=nmr[:, t : t + 1],
                            scale=rstd[:, t : t + 1],
                        )
                    sl = slice(4 * half, 4 * half + 4)
                    nc.sync.dma_start(outv[:, sl], OUT_sb[:, sl])

            # ---- emission schedule (per-engine queues are in-order!) ----
            P = (nc.gpsimd, nc.gpsimd)
            V = (nc.vector, nc.vector)
            xsum(0)
            accum(0, 0, P)
            matvec(0, 1)
            # xsum(1) sits in the PE queue after b0p1 so b0's first pass
            # never waits on the XT1 transfer.
            xsum(1)
            accum(0, 1, P)
            matvec(0, 2)
            accum(0, 2, P)
            matvec(0, 3)
            accum(0, 3, P)
            accum(1, 0, V)
            matvec(1, 1)
            accum(1, 1, V)
            s0 = sops(0)
            matvec(1, 2)
            accum(1, 2, V)
            chain_tanh_store(0, *s0)
            matvec(1, 3)
            s1 = sops(1)
            # both halves on Pool: DVE must go straight from the colb copy
            # into the LN-stat chain (the tail's longest pole)
            accum(1, 3, (nc.gpsimd, nc.gpsimd))
            chain_tanh_store(1, *s1)

    nc.compile()
    return nc


def _get_module():
    global _NC
    if _NC is None:
        _NC = _build_module()
    return _NC


def _make_in_maps(A, X, h):
    AT = np.ascontiguousarray(A.transpose(0, 2, 1)).astype(np.float16)
    XT = np.ascontiguousarray(X.transpose(0, 2, 1)).astype(np.float16)
    # fold the per-term 16^i (device stores P_i * 16^-i) into h, quantize,
    # and build the LN-stat Gram vectors from the *quantized* h so the
    # device stats match the device Y.
    hf = (
        (h * (SCALE ** np.arange(K1, dtype=np.float32))[:, None, None])
        .astype(np.float16)
        .astype(np.float32)
    )  # [K1, C, N]
    HT = np.ascontiguousarray(hf.transpose(2, 1, 0)).reshape(N, C * K1)
    HS = hf.sum(axis=1).T  # [N, K1]
    G = np.einsum("icn,jcn->nij", hf, hf).reshape(N, K1 * K1)  # [N, 16]
    # pack [HS | G] in column layout [p, t, 20]: row n = 128t + p
    HSG = np.concatenate([HS, G], axis=1).reshape(8, 128, K1 + K1 * K1)
    HSG = np.ascontiguousarray(HSG.transpose(1, 0, 2)).astype(np.float32)
    in_maps = []
    for core in range(NCORES):
        sl = slice(BPC * core, BPC * (core + 1))
        in_maps.append(
            {
                "AT": np.ascontiguousarray(AT[sl]),
                "XT": np.ascontiguousarray(XT[sl]),
                "HT": HT.astype(np.float16),
                "HSG": HSG,
            }
        )
    return in_maps


def _numpy_fallback(A, X, h, ln_gamma, ln_beta):
    Xs = X.sum(-1)
    p = Xs
    powers = [Xs]
    for _ in range(K1 - 1):
        p = np.einsum("bnm,bm->bn", A, p)
        powers.append(p)
    P = np.stack(powers)
    Y = np.einsum("icn,ibn->bnc", h, P)
    mu = Y.mean(axis=-1, keepdims=True)
    var = Y.var(axis=-1, keepdims=True)
    Yn = (Y - mu) / np.sqrt(var + LN_EPS) * ln_gamma + ln_beta
    return np.tanh(Yn).astype(np.float32)


def _run(A, X, h, ln_gamma, ln_beta, trace=False):
    A = np.ascontiguousarray(np.asarray(A, dtype=np.float32))
    X = np.ascontiguousarray(np.asarray(X, dtype=np.float32))
    h = np.ascontiguousarray(np.asarray(h, dtype=np.float32))
    g = np.asarray(ln_gamma, dtype=np.float32)
    be = np.asarray(ln_beta, dtype=np.float32)

    if not (np.all(g == 1.0) and np.all(be == 0.0)):
        # device kernel folds the (identity) affine away; anything else is
        # handled on host
        return _numpy_fallback(A, X, h, g, be), None

    from concourse import bass_utils

    nc = _get_module()
    try:
        res = bass_utils.run_bass_kernel_spmd(
            nc, _make_in_maps(A, X, h), core_ids=list(range(NCORES)), trace=trace
        )
    except ModuleNotFoundError:
        # containers without the axon NTFF profile hook can't trace
        res = bass_utils.run_bass_kernel_spmd(
            nc, _make_in_maps(A, X, h), core_ids=list(range(NCORES)), trace=False
        )
    out = np.concatenate([np.asarray(r["OUT"]) for r in res.results], axis=0)
    return out.astype(np.float32), res.exec_time_ns


def kernel(A, X, h, ln_gamma, ln_beta):
    out, _ = _run(A, X, h, ln_gamma, ln_beta, trace=False)
    return out


def kernel_profiled(A, X, h, ln_gamma, ln_beta):
    return _run(A, X, h, ln_gamma, ln_beta, trace=True)


# revision 24
# speedup vs baseline: 1.0042x; 1.0042x over previous
"""NodeVarGraphConvolutionLayer on 8 TRN2 NeuronCores.

Math (see reference):
  Xs = X.sum(-1)                        [B, N]
  P0 = Xs;  P_i = A @ P_{i-1}           (3 batched matvecs, N=1024)
  Y[b,n,c] = sum_i h[i,c,n] * P_i[b,n]  [B, N, 64]
  out = tanh(LayerNorm_c(Y) * gamma + beta)

Sharding: data-parallel over batch. B=16 -> 2 batches per core.

Design (stationary-A, fp16):
  Precision: bf16 fails the 2e-2 gate (LayerNorm rescales nodes whose
  matvec cancels to O(1), so dot noise flips signs -> rel 5.5e-2); fp16
  with fp32 PSUM accumulation lands at rel ~9e-3. P_i grows ~30x per
  term, so pass i stores P_i * 16^-i (ACT copy, scale=1/16) and the
  matching 16^i is folded into h on the host.
  Matvec: A is the PE *stationary* operand. For chunk j, block u:
      stat[p, s] = AT[128j+p, 128u+s],  mov[p] = P_{i-1}[128j+p]
      => out[s] = P_i[128u+s] accumulated over j into PSUM bank u
  so results land directly in column layout (n = 128t + p) - no
  transpose pipeline. A matmul start zeroes a whole 2KB PSUM bank, so
  the 8 interleaved groups own one bank each (pt_all spans all 8).
  PE cost ~ the A-stream floor: 64 self-loading (128-row ldweights + 1
  moving row) tiles per pass.
  Xs is also computed on the PE (X^T tiles x ones), directly in column
  layout.
  Schedule: batch-sequential passes (b0: P1..P3 then b1) so only b0's
  pass 1 is DMA-gated and all of b0's epilogue hides under b1's
  matmuls. Queues: A chunks + OUT stores on SP hwdge, X^T on ACT hwdge,
  HS|G + h on the Pool SWDGE queue (the ACT SEQ's 1.2us-per-DMA issue
  cost would otherwise sit in front of the first col copy). The
  pass-gating colb copies run on DVE (lower PSUM latency, and off the
  tanh-heavy ACT queue); colf/tanh on ACT. Emission order is arranged
  so no in-order queue blocks a later-needed op behind a long wait.
  Epilogue: Yacc (+)= h_i*P_i per pass in the next pass's shadow (b0 and
  b1's last term on Pool, b1's others on DVE). LayerNorm stats come from
  host-precomputed Gram vectors of the quantized h
  (S1 = sum_i hs_i P_i, S2 = sum_ij G_ij P_i P_j, both tiny DVE ops off
  the stored P vectors) - they never wait on Yacc assembly. rstd =
  Quake seed + 1 Newton step; tanh is fused per-t on ACT via
  scale/bias (tanh(Y*rstd - mean*rstd)); fp16 output (host upcasts).
"""

import numpy as np

B, N, C, K1 = 16, 1024, 64, 4
NCORES = 8
BPC = B // NCORES  # batches per core
LN_EPS = 1e-5
SCALE = 16.0

_NC = None


def _build_module():
    from concourse import bacc, bass, tile, mybir

    f32 = mybir.dt.float32
    f16 = mybir.dt.float16
    i32 = mybir.dt.int32
    AX = mybir.AxisListType
    OP = mybir.AluOpType
    AF = mybir.ActivationFunctionType

    nc = bacc.Bacc(
        "TRN2",
        target_bir_lowering=False,
        debug=False,
        enable_asserts=False,
    )

    AT_d = nc.dram_tensor("AT", [BPC, N, N], f16, kind="ExternalInput").ap()
    XT_d = nc.dram_tensor("XT", [BPC, C, N], f16, kind="ExternalInput").ap()
    HT_d = nc.dram_tensor("HT", [N, C * K1], f16, kind="ExternalInput").ap()
    # HS|G packed, already in column layout [p, t, 4+16]
    HSG_d = nc.dram_tensor("HSG", [128, 8, K1 + K1 * K1], f32, kind="ExternalInput").ap()
    OUT_d = nc.dram_tensor("OUT", [BPC, N, C], f16, kind="ExternalOutput").ap()

    with tile.TileContext(nc) as tc:
        with (
            tc.tile_pool(name="big", bufs=2) as big,
            tc.tile_pool(name="aux", bufs=1) as aux,
            tc.tile_pool(name="psum", bufs=1, space="PSUM") as psum,
        ):
            # --- small loads on the ACT hwdge queue ----------------------
            XT_sbs = []
            for b in range(BPC):
                XT_sb = aux.tile([C, N], f16, tag=f"XT{b}", name=f"XT_sb{b}")
                nc.scalar.dma_start(XT_sb, XT_d[b])
                XT_sbs.append(XT_sb)
            # LN-stat vectors + h on the Pool SWDGE queue so the ACT SEQ
            # only issues the two X^T loads before the first col copies.
            HSG_sb = aux.tile([128, 8, K1 + K1 * K1], f32, tag="HSG")
            nc.gpsimd.dma_start(HSG_sb, HSG_d)
            # view [p, t, 5, 4]: row 0 = hs, rows 1..4 = G
            HSGv = HSG_sb.rearrange("p t (i j) -> p t i j", i=K1 + 1)
            H_sb = aux.tile([128, 8, C * K1], f16, tag="H")
            nc.gpsimd.dma_start(H_sb, HT_d.rearrange("(t p) x -> p t x", t=8, p=128))
            Hv = H_sb.rearrange("p t (c i) -> p t c i", c=C, i=K1)

            zero_sb = aux.tile([128, 1], f32, tag="zero")
            nc.vector.memset(zero_sb, 0.0)
            magic = aux.tile([128, 1], i32, tag="magic")
            nc.vector.memset(magic, 0x5F3759DF)
            magicb = magic.broadcast_to([128, 8])
            ones_sb = aux.tile([C, 1], f16, tag="ones")
            nc.vector.memset(ones_sb, 1.0)
            wz = aux.tile([128, 128], f16, tag="wz")
            nc.vector.memset(wz, 0.0)
            # Preload the Tanh ACT table while DMAs run.
            warm = aux.tile([128, 1], f32, tag="warm")
            nc.scalar.activation(warm, zero_sb, AF.Tanh, bias=zero_sb)

            # pt_all spans all 8 PSUM banks: a matmul start zeroes a whole
            # 2KB bank region, so each interleaved group owns a bank.
            pt_all = psum.tile([128, 8, 512], f32, tag="pt", name="pt")
            # PE p-state warmup under the initial DMA window.
            for _ in range(16):
                nc.tensor.matmul(
                    pt_all[:, 0, 0:1], wz, wz[:, 0:1], start=True, stop=True
                )

            # --- A: 16 chunk DMAs on the SP hwdge queue ------------------
            # A_sb[b][j][p, m] = AT[b, 128j+p, m]
            # b0: 8 chunk DMAs (pass 1 streams them as they land); b1: one
            # big DMA (pass 1 for b1 starts late enough anyway) to cut
            # SEQ/HWDGE issue overhead.
            A0_sbs = [
                aux.tile([128, N], f16, tag=f"A0j{j}", name=f"A0j{j}")
                for j in range(8)
            ]
            for j in range(8):
                nc.sync.dma_start(A0_sbs[j], AT_d[0, 128 * j : 128 * (j + 1), :])
            A1_sb = aux.tile([128, 8, N], f16, tag="A1", name="A1_sb")
            for h in range(2):
                nc.sync.dma_start(
                    A1_sb[:, 4 * h : 4 * h + 4, :],
                    AT_d[1, 512 * h : 512 * h + 512, :].rearrange(
                        "(j p) m -> p j m", j=4, p=128
                    ),
                )
            A_sbs = [A0_sbs, [A1_sb[:, j, :] for j in range(8)]]

            # colb[p, t, i] = P_i[128t+p] * 16^-i (fp16; PE rhs + h-product
            # input); colf = f32 upcast of colb for the LN-stat ops.
            colbs, colfs, Yaccs = [], [], []
            for b in range(BPC):
                colbs.append(
                    big.tile([128, 8, K1], f16, tag=f"colb{b}", name=f"colb{b}")
                )
                colfs.append(
                    big.tile([128, 8, K1], f32, tag=f"colf{b}", name=f"colf{b}")
                )
                Yaccs.append(
                    big.tile([128, 8, C], f32, tag=f"Yacc{b}", name=f"Yacc{b}")
                )

            def xsum(b):
                # P_0 = X.sum(-1) on the PE: out[s] = sum_c XT[c, 128u+s],
                # straight into column layout.
                for u in range(8):
                    nc.tensor.matmul(
                        pt_all[:, u, 0:1],
                        XT_sbs[b][:, 128 * u : 128 * (u + 1)],
                        ones_sb,
                        start=True,
                        stop=True,
                    )
                nc.vector.tensor_scalar_mul(colbs[b][:, :, 0], pt_all[:, :, 0], 1.0)
                nc.scalar.copy(colfs[b][:, :, 0], colbs[b][:, :, 0])

            def matvec(b, i):
                # Pass 1 runs j-outer (each PSUM group consumes chunk j as
                # its DMA lands). Resident passes (i>=2) run u-outer with a
                # per-bank colb copy as each group closes: the NEXT u-outer
                # pass then finds every colb element and PSUM-bank WAR
                # already satisfied when the PE reaches it - zero boundary
                # bubble, and the PE p-state never cools between passes.
                colb = colbs[b]
                if i == 1:
                    for j in range(8):
                        A_j = A_sbs[b][j]
                        mov = colb[:, j, i - 1 : i]
                        for u in range(8):
                            nc.tensor.matmul(
                                pt_all[:, u, 0:1],
                                A_j[:, 128 * u : 128 * (u + 1)],
                                mov,
                                start=(j == 0),
                                stop=(j == 7),
                            )
                    nc.vector.tensor_scalar_mul(
                        colb[:, :, i], pt_all[:, :, 0], 1.0 / SCALE
                    )
                    nc.scalar.copy(colfs[b][:, :, i], colb[:, :, i])
                    return
                for u in range(8):
                    for j in range(8):
                        nc.tensor.matmul(
                            pt_all[:, u, 0:1],
                            A_sbs[b][j][:, 128 * u : 128 * (u + 1)],
                            colb[:, j, i - 1 : i],
                            start=(j == 0),
                            stop=(j == 7),
                        )
                    nc.vector.tensor_scalar_mul(
                        colb[:, u : u + 1, i], pt_all[:, u, 0:1], 1.0 / SCALE
                    )
                if i == K1 - 1:
                    # colf (feeds the LN-stat chain, the tail's pole) straight
                    # from PSUM on DVE right after the last group closes
                    nc.vector.tensor_scalar_mul(
                        colfs[b][:, :, i], pt_all[:, :, 0], 1.0 / SCALE
                    )
                else:
                    nc.scalar.copy(colfs[b][:, :, i], colb[:, :, i])

            def accum(b, i, engs):
                # Yacc[:, t, c] (+)= h_i[c, n] * P_i[n]; one engine per
                # t-half (pass the same engine twice to run full-width).
                colb = colbs[b]
                Yacc = Yaccs[b]
                for half, eng in enumerate(engs):
                    sl = slice(4 * half, 4 * half + 4)
                    pb = colb[:, sl, i].unsqueeze(2).broadcast_to([128, 4, C])
                    if i == 0:
                        eng.tensor_tensor(Yacc[:, sl], Hv[:, sl, :, 0], pb, OP.mult)
                    else:
                        tmpE = big.tile(
                            [128, 8, C],
                            f32,
                            tag=f"tmpE{b}i{i}",
                            name=f"tmpE{b}i{i}h{half}",
                        )
                        eng.tensor_tensor(tmpE[:, sl], Hv[:, sl, :, i], pb, OP.mult)
                        eng.tensor_tensor(Yacc[:, sl], Yacc[:, sl], tmpE[:, sl], OP.add)

            def sops(b):
                # LN stats straight from the stored P vectors, via one
                # augmented-Gram product GPa[r, j] = HSGv[r, j] * P_j:
                #   S1 = sum_j GPa[0, j]           (hs row)
                #   S2 = sum_ij GPa[1+i, j] * P_i  (Gram rows)
                colf = colfs[b]
                GPa = big.tile(
                    [128, 8, K1 + 1, K1], f32, tag=f"GPa{b}", name=f"GPa{b}"
                )
                nc.vector.tensor_tensor(
                    GPa,
                    HSGv,
                    colf.unsqueeze(2).broadcast_to([128, 8, K1 + 1, K1]),
                    OP.mult,
                )
                S1 = big.tile([128, 8], f32, tag=f"S1{b}", name=f"S1{b}")
                nc.vector.tensor_reduce(S1, GPa[:, :, 0, :], AX.X, OP.add)
                PP = big.tile([128, 8, K1, K1], f32, tag=f"PP{b}", name=f"PP{b}")
                nc.vector.tensor_tensor(
                    PP,
                    GPa[:, :, 1:, :],
                    colf.unsqueeze(3).broadcast_to([128, 8, K1, K1]),
                    OP.mult,
                )
                S2 = big.tile([128, 8], f32, tag=f"S2{b}", name=f"S2{b}")
                nc.vector.tensor_reduce(S2, PP, AX.XY, OP.add)
                return S1, S2

            def chain_tanh_store(b, S1, S2):
                # mean/var -> rstd (Quake + 1 Newton) -> fused tanh -> store
                # veps = S2/64 + eps - (S1/64)^2, with the mean folded into
                # the nmr op so it is off the rstd critical path
                mse = big.tile([128, 8], f32, tag=f"mse{b}", name=f"mse{b}")
                nc.vector.tensor_scalar(mse, S2, 1.0 / C, LN_EPS, OP.mult, OP.add)
                m2 = big.tile([128, 8], f32, tag=f"m2{b}", name=f"m2{b}")
                nc.vector.tensor_tensor(m2, S1, S1, OP.mult)
                veps = big.tile([128, 8], f32, tag=f"veps{b}", name=f"veps{b}")
                nc.vector.scalar_tensor_tensor(
                    veps, m2, -1.0 / (C * C), mse, OP.mult, OP.add
                )

                rstd = big.tile([128, 8], f32, tag=f"rstd{b}", name=f"rstd{b}")
                nc.vector.tensor_scalar(
                    rstd.bitcast(i32),
                    veps.bitcast(i32),
                    1,
                    None,
                    OP.logical_shift_right,
                )
                nc.vector.tensor_tensor(
                    rstd.bitcast(i32), magicb, rstd.bitcast(i32), OP.subtract
                )
                tq = big.tile([128, 8], f32, tag=f"tq{b}", name=f"tq{b}")
                nc.vector.tensor_tensor(tq, rstd, rstd, OP.mult)
                nc.vector.scalar_tensor_tensor(
                    tq, tq, -0.5, veps, OP.mult, OP.mult
                )
                nc.vector.scalar_tensor_tensor(
                    rstd, tq, 1.5, rstd, OP.add, OP.mult
                )
                nmr = big.tile([128, 8], f32, tag=f"nmr{b}", name=f"nmr{b}")
                nc.vector.scalar_tensor_tensor(
                    nmr, S1, -1.0 / C, rstd, OP.mult, OP.mult
                )

                # tanh(Yacc*rstd + nmr) per t on ACT; store halves on SP
                Yacc = Yaccs[b]
                OUT_sb = big.tile([128, 8, C], f16, tag=f"OUTS{b}", name=f"OUTS{b}")
                outv = OUT_d[b].rearrange("(t p) c -> p t c", t=8, p=128)
                for half in range(2):
                    for t in range(4 * half, 4 * half + 4):
                        nc.scalar.activation(
                            OUT_sb[:, t],
                            Yacc[:, t],
                            AF.Tanh,
                            bias=nmr[:, t : t + 1],
                            scale=rstd[:, t : t + 1],
                        )
                    sl = slice(4 * half, 4 * half + 4)
                    nc.sync.dma_start(outv[:, sl], OUT_sb[:, sl])

            # ---- emission schedule (per-engine queues are in-order!) ----
            P = (nc.gpsimd, nc.gpsimd)
            V = (nc.vector, nc.vector)
            xsum(0)
            accum(0, 0, P)
            matvec(0, 1)
            # xsum(1) sits in the PE queue after b0p1 so b0's first pass
            # never waits on the XT1 transfer.
            xsum(1)
            accum(0, 1, P)
            matvec(0, 2)
            accum(0, 2, P)
            matvec(0, 3)
            accum(0, 3, P)
            accum(1, 0, V)
            matvec(1, 1)
            accum(1, 1, V)
            s0 = sops(0)
            matvec(1, 2)
            accum(1, 2, V)
            chain_tanh_store(0, *s0)
            matvec(1, 3)
            s1 = sops(1)
            # both halves on Pool: DVE must go straight from the colb copy
            # into the LN-stat chain (the tail's longest pole)
            accum(1, 3, (nc.gpsimd, nc.gpsimd))
            chain_tanh_store(1, *s1)

    nc.compile()
    return nc


def _get_module():
    global _NC
    if _NC is None:
        _NC = _build_module()
    return _NC


def _make_in_maps(A, X, h):
    AT = np.ascontiguousarray(A.transpose(0, 2, 1)).astype(np.float16)
    XT = np.ascontiguousarray(X.transpose(0, 2, 1)).astype(np.float16)
    # fold the per-term 16^i (device stores P_i * 16^-i) into h, quantize,
    # and build the LN-stat Gram vectors from the *quantized* h so the
    # device stats match the device Y.
    hf = (
        (h * (SCALE ** np.arange(K1, dtype=np.float32))[:, None, None])
        .astype(np.float16)
        .astype(np.float32)
    )  # [K1, C, N]
    HT = np.ascontiguousarray(hf.transpose(2, 1, 0)).reshape(N, C * K1)
    HS = hf.sum(axis=1).T  # [N, K1]
    G = np.einsum("icn,jcn->nij", hf, hf).reshape(N, K1 * K1)  # [N, 16]
    # pack [HS | G] in column layout [p, t, 20]: row n = 128t + p
    HSG = np.concatenate([HS, G], axis=1).reshape(8, 128, K1 + K1 * K1)
    HSG = np.ascontiguousarray(HSG.transpose(1, 0, 2)).astype(np.float32)
    in_maps = []
    for core in range(NCORES):
        sl = slice(BPC * core, BPC * (core + 1))
        in_maps.append(
            {
                "AT": np.ascontiguousarray(AT[sl]),
                "XT": np.ascontiguousarray(XT[sl]),
                "HT": HT.astype(np.float16),
                "HSG": HSG,
            }
        )
    return in_maps


def _numpy_fallback(A, X, h, ln_gamma, ln_beta):
    Xs = X.sum(-1)
    p = Xs
    powers = [Xs]
    for _ in range(K1 - 1):
        p = np.einsum("bnm,bm->bn", A, p)
        powers.append(p)
    P = np.stack(powers)
    Y = np.einsum("icn,ibn->bnc", h, P)
    mu = Y.mean(axis=-1, keepdims=True)
    var = Y.var(axis=-1, keepdims=True)
    Yn = (Y - mu) / np.sqrt(var + LN_EPS) * ln_gamma + ln_beta
    return np.tanh(Yn).astype(np.float32)


def _run(A, X, h, ln_gamma, ln_beta, trace=False):
    A = np.ascontiguousarray(np.asarray(A, dtype=np.float32))
    X = np.ascontiguousarray(np.asarray(X, dtype=np.float32))
    h = np.ascontiguousarray(np.asarray(h, dtype=np.float32))
    g = np.asarray(ln_gamma, dtype=np.float32)
    be = np.asarray(ln_beta, dtype=np.float32)

    if not (np.all(g == 1.0) and np.all(be == 0.0)):
        # device kernel folds the (identity) affine away; anything else is
        # handled on host
        return _numpy_fallback(A, X, h, g, be), None

    from concourse import bass_utils

    nc = _get_module()
    try:
        res = bass_utils.run_bass_kernel_spmd(
            nc, _make_in_maps(A, X, h), core_ids=list(range(NCORES)), trace=trace
        )
    except ModuleNotFoundError:
        # containers without the axon NTFF profile hook can't trace
        res = bass_utils.run_bass_kernel_spmd(
            nc, _make_in_maps(A, X, h), core_ids=list(range(NCORES)), trace=False
        )
    out = np.concatenate([np.asarray(r["OUT"]) for r in res.results], axis=0)
    return out.astype(np.float32), res.exec_time_ns


def kernel(A, X, h, ln_gamma, ln_beta):
    out, _ = _run(A, X, h, ln_gamma, ln_beta, trace=False)
    return out


def kernel_profiled(A, X, h, ln_gamma, ln_beta):
    return _run(A, X, h, ln_gamma, ln_beta, trace=True)
